# revision 1
# baseline (speedup 1.0000x reference)
"""CAMMambaBlock Trainium2 kernel.

Data-parallel over batch: 8 batch elements -> 8 NeuronCores. Each core runs
the full block (LayerNorm -> in_proj -> causal depthwise conv -> SiLU ->
x_proj -> dt softplus -> selective scan -> gating -> out_proj -> residual)
on its own (c=128, L=9216) slice, streaming over L in chunks.

Selective scan runs on the native DVE prefix-scan instruction
(tensor_tensor_scan: state = dA*state + u), one recurrence per (c, n) pair,
16 state tiles per chunk, chained across chunks via `initial` APs.
B/C coefficients are broadcast across partitions via DRAM-bounce DMA
(the only engine-free replication path), in bf16 so the surrounding
elementwise work hits the DVE 2x mode.
"""
import types
import numpy as np
import ml_dtypes
from contextlib import ExitStack

import bass_rust

import concourse.bass as bass
import concourse.bacc as bacc
import concourse.tile as tile
from concourse import mybir
from concourse.bass_utils import run_bass_kernel_spmd
from concourse.hw_specs import get_activation_tables


def _single_act_table(self):
    """Force every activation onto natural_log_exp_and_others so the
    table-load pass hoists to one load (the greedy per-func picker would
    otherwise alternate sets and reload ~2.7us each time)."""
    if not any(i.opcode == "Activation" for i in self.all_instructions()):
        return
    keep = "natural_log_exp_and_others"
    tables = [(n, (f if n == keep else set()))
              for n, f in get_activation_tables(self.m.arch).items()]
    bass_rust.insert_act_table_loads(self, tables)

F32 = mybir.dt.float32
F32R = mybir.dt.float32r
BF16 = mybir.dt.bfloat16
AF = mybir.ActivationFunctionType
OP = mybir.AluOpType

C = 128           # channels == d_inner == partitions
NSTATE = 16       # SSM state dim
RANK = 8          # dt rank
LN_EPS = 1e-5
DCONV = 4

L_FULL = 96 * 96  # 9216


def build_nc(L, Tc, sub=512):
    """Build the single-core Bass graph (SPMD across cores)."""
    assert L % Tc == 0 and Tc % sub == 0
    nchunk = L // Tc
    nsub = Tc // sub

    nc = bacc.Bacc()
    x_in = nc.declare_dram_parameter("x", [C, L], F32, isOutput=False)
    w_inT = nc.declare_dram_parameter("w_inT", [C, 5 * C], BF16, isOutput=False)
    w_xpT = nc.declare_dram_parameter("w_xpT", [C, RANK + 2 * NSTATE], BF16,
                                      isOutput=False)
    w_dtT = nc.declare_dram_parameter("w_dtT", [RANK, C], BF16, isOutput=False)
    w_outT = nc.declare_dram_parameter("w_outT", [C, C], BF16, isOutput=False)
    # per-partition columns: [ln_w, ln_b, conv_b, dt_b, D, conv_w0..3, eps]
    cols = nc.declare_dram_parameter("cols", [C, 11], F32, isOutput=False)
    a_cols = nc.declare_dram_parameter("a_cols", [C, NSTATE], F32,
                                       isOutput=False)
    y_out = nc.declare_dram_parameter("y", [C, L], F32, isOutput=True)

    with tile.TileContext(nc) as tc, ExitStack() as ctx:
        wpool = ctx.enter_context(tc.tile_pool(name="weights", bufs=1))
        state = ctx.enter_context(tc.tile_pool(name="state", bufs=1))
        io = ctx.enter_context(tc.tile_pool(name="io", bufs=2))
        work = ctx.enter_context(tc.tile_pool(name="work", bufs=2))
        scanp = ctx.enter_context(tc.tile_pool(name="scan", bufs=4))
        treep = ctx.enter_context(tc.tile_pool(name="tree", bufs=1))
        scr = ctx.enter_context(tc.tile_pool(name="scratch", bufs=1))
        dram = ctx.enter_context(tc.tile_pool(name="dram", bufs=2,
                                              space="DRAM"))
        ps_st = ctx.enter_context(tc.tile_pool(name="ps_st", bufs=1,
                                               space="PSUM"))
        ps_mm = ctx.enter_context(tc.tile_pool(name="ps_mm", bufs=1,
                                               space="PSUM"))

        # ---- weights to SBUF (once) ----
        winT = wpool.tile([C, 5 * C], BF16, tag="winT")
        nc.sync.dma_start(winT[:], w_inT[:])
        wxpT = wpool.tile([C, RANK + 2 * NSTATE], BF16, tag="wxpT")
        nc.sync.dma_start(wxpT[:], w_xpT[:])
        wdtT = wpool.tile([RANK, C], BF16, tag="wdtT")
        nc.sync.dma_start(wdtT[:], w_dtT[:])
        woutT = wpool.tile([C, C], BF16, tag="woutT")
        nc.sync.dma_start(woutT[:], w_outT[:])
        colsb = wpool.tile([C, 11], F32, tag="cols")
        nc.sync.dma_start(colsb[:], cols[:])
        acol = wpool.tile([C, NSTATE], F32, tag="acol")
        nc.sync.dma_start(acol[:], a_cols[:])
        ones_c = wpool.tile([C, C], BF16, tag="ones")
        nc.gpsimd.memset(ones_c[:], 1.0 / C)

        ln_w, ln_b = colsb[:, 0:1], colsb[:, 1:2]
        conv_b, dt_b, d_col = colsb[:, 2:3], colsb[:, 3:4], colsb[:, 4:5]

        # ---- persistent state ----

        carries = []
        for n in range(NSTATE):
            t = state.tile([C, 1], BF16, tag=f"carry{n}")
            carries.append(t)

        # ---- streaming loop ----
        for k in range(nchunk):
            t0 = k * Tc
            xin = io.tile([C, Tc], F32, tag="xin")
            nc.sync.dma_start(xin[:], x_in[:, t0:t0 + Tc])

            # LayerNorm over channel (partition) dim
            xin_bf = scr.tile([C, Tc], BF16, tag="xinbf")
            nc.gpsimd.dma_start(xin_bf[:], x_in[:, t0:t0 + Tc])
            sq = scr.tile([C, Tc], BF16, tag="sq")
            nc.scalar.activation(sq[:], xin[:], AF.Square)
            un = work.tile([C, Tc + DCONV - 1], BF16, tag="un")
            if k == 0:
                nc.vector.memset(un[:, 0:DCONV - 1], 0.0)
            else:
                nc.vector.tensor_copy(un[:, 0:DCONV - 1],
                                      prev_un[:, Tc:Tc + DCONV - 1])
            prev_un = un
            for j in range(nsub):
                sl = slice(j * sub, (j + 1) * sub)
                mu = ps_st.tile([C, sub], F32, tag="mu")
                nc.tensor.matmul(mu[:], ones_c[:],
                                 xin_bf[:, sl],
                                 start=True, stop=True)
                m2 = ps_st.tile([C, sub], F32, tag="m2")
                nc.tensor.matmul(m2[:], ones_c[:],
                                 sq[:, sl],
                                 start=True, stop=True)
                musq = scr.tile([C, sub], F32, tag="musq")
                nc.scalar.activation(musq[:], mu[:], AF.Square)
                var = scr.tile([C, sub], F32, tag="var")
                nc.vector.tensor_tensor(var[:], m2[:], musq[:], OP.subtract)
                lnv = scr.tile([C, sub], F32, tag="lnv")
                nc.scalar.activation(lnv[:], var[:], AF.Ln, bias=colsb[:, 9:10])
                rstd = scr.tile([C, sub], F32, tag="rstd")
                nc.scalar.activation(rstd[:], lnv[:], AF.Exp, scale=-0.5)
                dmu = scr.tile([C, sub], F32, tag="dmu")
                nc.vector.tensor_tensor(dmu[:], xin[:, sl], mu[:], OP.subtract)
                nc.vector.tensor_tensor(dmu[:], dmu[:], rstd[:], OP.mult)
                nc.vector.tensor_scalar(
                    un[:, DCONV - 1 + j * sub:DCONV - 1 + (j + 1) * sub],
                    dmu[:], ln_w, ln_b, OP.mult, OP.add)

            # in_proj + folded causal conv (4 shifted matmuls accumulate),
            # z -> zs = silu(z); xs = silu(conv + conv_b) via STT
            zs = work.tile([C, Tc], F32, tag="zs")
            xs = work.tile([C, Tc], BF16, tag="xs")
            for j in range(nsub):
                sl = slice(j * sub, (j + 1) * sub)
                xm_ps = ps_mm.tile([C, sub], F32, tag="xm_ps")
                for kk in range(DCONV):
                    nc.tensor.matmul(
                        xm_ps[:], winT[:, kk * C:(kk + 1) * C],
                        un[:, kk + j * sub:kk + j * sub + sub],
                        start=(kk == 0), stop=(kk == DCONV - 1))
                z_ps = ps_mm.tile([C, sub], F32, tag="z_ps")
                nc.tensor.matmul(z_ps[:], winT[:, 4 * C:5 * C],
                                 un[:, DCONV - 1 + j * sub:
                                     DCONV - 1 + j * sub + sub],
                                 start=True, stop=True)
                es1 = scr.tile([C, sub], F32, tag="es1")
                nc.scalar.activation(es1[:], z_ps[:], AF.Exp, scale=-1.0)
                es2 = scr.tile([C, sub], F32, tag="es2")
                nc.scalar.activation(es2[:], es1[:], AF.Ln, bias=1.0)
                sgz = scr.tile([C, sub], F32, tag="sgz")
                nc.scalar.activation(sgz[:], es2[:], AF.Exp, scale=-1.0)
                nc.vector.tensor_tensor(zs[:, sl], z_ps[:], sgz[:], OP.mult)
                # silu(conv + cb): e^{-(x+cb)} -> ln1p -> e^{-.} -> (x+cb)*sg
                ec1 = scr.tile([C, sub], F32, tag="ec1")
                nc.scalar.activation(ec1[:], xm_ps[:], AF.Exp, scale=-1.0,
                                     bias=colsb[:, 10:11])
                ec2 = scr.tile([C, sub], F32, tag="ec2")
                nc.scalar.activation(ec2[:], ec1[:], AF.Ln, bias=1.0)
                sgc = scr.tile([C, sub], F32, tag="ec1")
                nc.scalar.activation(sgc[:], ec2[:], AF.Exp, scale=-1.0)
                nc.vector.scalar_tensor_tensor(xs[:, sl], xm_ps[:], conv_b,
                                               sgc[:], OP.add, OP.mult)

            # x_proj -> dtr rows + B/C rows (bf16, bounced to DRAM)
            dtr = work.tile([RANK, Tc], BF16, tag="dtr")
            bc = work.tile([2 * NSTATE, Tc], BF16, tag="bc")
            for j in range(nsub):
                sl = slice(j * sub, (j + 1) * sub)
                dbl = ps_mm.tile([RANK + 2 * NSTATE, sub], F32, tag="dbl")
                nc.tensor.matmul(dbl[:], wxpT[:],
                                 xs[:, sl],
                                 start=True, stop=True)
                nc.scalar.copy(bc[:, sl], dbl[0:2 * NSTATE, :])
                nc.scalar.copy(dtr[:, sl],
                               dbl[2 * NSTATE:2 * NSTATE + RANK, :])
            bcd = dram.tile([NSTATE, 2 * Tc], BF16, tag="bcd")
            nc.sync.dma_start(bcd[:], bc[:])

            # dt = softplus(dt_proj @ dtr + dt_b)
            dt_sb = work.tile([C, Tc], F32, tag="dt")
            for j in range(nsub):
                sl = slice(j * sub, (j + 1) * sub)
                dt_ps = ps_mm.tile([C, sub], F32, tag="dt_ps")
                nc.tensor.matmul(dt_ps[:], wdtT[:],
                                 dtr[:, sl],
                                 start=True, stop=True)
                spe = scr.tile([C, sub], F32, tag="spe")
                nc.scalar.activation(spe[:], dt_ps[:], AF.Exp, bias=dt_b)
                nc.scalar.activation(dt_sb[:, sl], spe[:], AF.Ln, bias=1.0)

            # v = dt * xs (bf16)
            v_bf = work.tile([C, Tc], BF16, tag="v")
            nc.gpsimd.tensor_tensor(v_bf[:], dt_sb[:], xs[:], OP.mult)

            # per-state scan + readout; B/C broadcast via DRAM-bounce DMAs
            ps = []
            for n in range(NSTATE):
                dA = scanp.tile([C, Tc], BF16, tag="dA")
                nc.scalar.activation(dA[:], dt_sb[:], AF.Exp,
                                     scale=acol[:, n:n + 1])
                bcr = scanp.tile([C, 2 * Tc], BF16, tag="bcr")
                nc.sync.dma_start(
                    bcr[:], bcd[n:n + 1, :].broadcast_to([C, 2 * Tc]))
                brep = bcr[:, 0:Tc]
                u = scanp.tile([C, Tc], BF16, tag="u")
                if n < 7:
                    nc.gpsimd.tensor_tensor(u[:], v_bf[:], brep[:], OP.mult)
                else:
                    nc.vector.tensor_tensor(u[:], v_bf[:], brep[:], OP.mult)
                if n < 8:
                    h = treep.tile([C, Tc], BF16, tag=f"p{n}")
                else:
                    h = scanp.tile([C, Tc], BF16, tag="ptmp")
                init = 0.0 if k == 0 else carries[n][:]
                nc.vector.tensor_tensor_scan(h[:], dA[:], u[:], init,
                                             OP.mult, OP.add)
                nc.vector.tensor_copy(carries[n][:], h[:, Tc - 1:Tc])
                crep = bcr[:, Tc:2 * Tc]
                if n < 7:
                    nc.gpsimd.tensor_tensor(h[:], h[:], crep[:], OP.mult)
                else:
                    nc.vector.tensor_tensor(h[:], h[:], crep[:], OP.mult)
                if n >= 8:
                    if n < 12:
                        nc.gpsimd.dma_start(ps[n - 8][:], h[:],
                                            accum_op=OP.add)
                    elif n < 14:
                        nc.gpsimd.tensor_tensor(ps[n - 8][:], ps[n - 8][:],
                                                h[:], OP.add)
                    else:
                        nc.vector.tensor_tensor(ps[n - 8][:], ps[n - 8][:],
                                                h[:], OP.add)
                else:
                    ps.append(h)
            # in-place tree reduction of 8 p tiles (bf16)
            for j, i in enumerate(range(0, 8, 2)):
                if j < 2:
                    nc.gpsimd.tensor_tensor(ps[i][:], ps[i][:], ps[i + 1][:],
                                            OP.add)
                else:
                    nc.vector.tensor_tensor(ps[i][:], ps[i][:], ps[i + 1][:],
                                            OP.add)
            for stride in (2, 4):
                for i in range(0, 8, 2 * stride):
                    nc.vector.tensor_tensor(ps[i][:], ps[i][:],
                                            ps[i + stride][:], OP.add)

            # y = tree + D*xs ; gate with zs
            y = scr.tile([C, Tc], F32, tag="y")
            nc.vector.scalar_tensor_tensor(y[:], xs[:], d_col, ps[0][:],
                                           OP.mult, OP.add)
            yg = scr.tile([C, Tc], BF16, tag="yg")
            nc.gpsimd.tensor_tensor(yg[:], y[:], zs[:], OP.mult)

            # out_proj + residual
            for j in range(nsub):
                sl = slice(j * sub, (j + 1) * sub)
                o_ps = ps_mm.tile([C, sub], F32, tag="o_ps")
                nc.tensor.matmul(o_ps[:], woutT[:],
                                 yg[:, sl],
                                 start=True, stop=True)
                ob = io.tile([C, sub], F32, tag="ob")
                nc.vector.tensor_tensor(ob[:], o_ps[:], xin[:, sl], OP.add)
                nc.sync.dma_start(y_out[:, t0 + j * sub:t0 + (j + 1) * sub],
                                  ob[:])
    nc.insert_act_table_loads = types.MethodType(_single_act_table, nc)
    nc.compile()
    return nc


def prep_weights(ln_w, ln_b, in_proj_w, conv_w, conv_b, x_proj_w,
                 dt_proj_w, dt_proj_b, A_log, D, out_proj_w):
    eps = np.full_like(ln_w, LN_EPS)
    cols = np.stack([ln_w, ln_b, conv_b, dt_proj_b, D,
                     conv_w[:, 0], conv_w[:, 1], conv_w[:, 2], conv_w[:, 3],
                     eps, -conv_b], axis=1).astype(np.float32)
    return {
        "w_inT": np.ascontiguousarray(np.concatenate(
            [in_proj_w[:128].T * conv_w[:, kk][None, :]
             for kk in range(4)] + [in_proj_w[128:].T],
            axis=1).astype(ml_dtypes.bfloat16)),
        "w_xpT": np.ascontiguousarray(
            x_proj_w[[8 + (i // 2) + 16 * (i % 2) for i in range(32)]
                     + list(range(8))].T
            .astype(ml_dtypes.bfloat16)),
        "w_dtT": np.ascontiguousarray(dt_proj_w.T.astype(ml_dtypes.bfloat16)),
        "w_outT": np.ascontiguousarray(
            out_proj_w.T.astype(ml_dtypes.bfloat16)),
        "cols": cols,
        "a_cols": np.ascontiguousarray(-np.exp(A_log.astype(np.float32))),
    }


def kernel(input, ln_w, ln_b, in_proj_w, conv_w, conv_b, x_proj_w,
           dt_proj_w, dt_proj_b, A_log, D, out_proj_w, _run=None):
    input = np.asarray(input, np.float32)
    b, c, H, W = input.shape
    L = H * W
    assert c == C and b == 8
    wts = prep_weights(
        np.asarray(ln_w, np.float32), np.asarray(ln_b, np.float32),
        np.asarray(in_proj_w, np.float32), np.asarray(conv_w, np.float32),
        np.asarray(conv_b, np.float32), np.asarray(x_proj_w, np.float32),
        np.asarray(dt_proj_w, np.float32), np.asarray(dt_proj_b, np.float32),
        np.asarray(A_log, np.float32), np.asarray(D, np.float32),
        np.asarray(out_proj_w, np.float32))
    nc = build_nc(L, 1536, 512)
    in_maps = []
    for i in range(8):
        m = {"x": np.ascontiguousarray(input[i].reshape(c, L))}
        m.update(wts)
        in_maps.append(m)
    run = _run or run_bass_kernel_spmd
    res = run(nc, in_maps, core_ids=list(range(8)))
    out = np.stack([np.asarray(res.results[i]["y"]).reshape(c, H, W)
                    for i in range(8)])
    return out.astype(np.float32)



# revision 4
# speedup vs baseline: 1.5272x; 1.5272x over previous
"""CAMMambaBlock Trainium2 kernel, v2 (state-interleaved scan layout).

Data-parallel over batch: 8 batch elements -> 8 NeuronCores. Each core runs
the full block (LayerNorm -> in_proj -> causal depthwise conv -> SiLU ->
x_proj -> dt softplus -> selective scan -> gating -> out_proj -> residual)
on its own (c=128, L=9216) slice, streaming over L in chunks of 1536.

Key layout: the selective scan runs in a state-interleaved layout.  For each
of 16 channel-groups g (8 channels each), a [128, Tc] tile holds all 16 SSM
states: partition p = n*8 + c8 carries the recurrence for (state n, channel
8g+c8).  This makes the per-state B/C coefficient broadcast a single shared
replicated tile per chunk (instead of 16 per-state broadcasts), and turns
the sum over states into PE selector-matmuls that accumulate in PSUM
(instead of a DVE/GpSimd reduction tree).

GpSimd is parked (it shares the DVE's second SBUF port; the v1 kernel's
heavy GpSimd elementwise load degraded DVE scans/TTs by 1.2-2x).  All
elementwise work is DVE in bf16 (2x mode); transcendentals on Scalar;
reductions and projections on the PE.  LayerNorm's ln_w/ln_b and the conv
bias are folded into the in_proj weights host-side.
"""
import types
import numpy as np
import ml_dtypes
from contextlib import ExitStack

import bass_rust

import concourse.bass as bass
import concourse.bacc as bacc
import concourse.tile as tile
from concourse import mybir
from concourse.bass_utils import run_bass_kernel_spmd
from concourse.hw_specs import get_activation_tables


def _single_act_table(self):
    """Force every activation onto natural_log_exp_and_others so the
    table-load pass hoists to one load."""
    if not any(i.opcode == "Activation" for i in self.all_instructions()):
        return
    keep = "natural_log_exp_and_others"
    tables = [(n, (f if n == keep else set()))
              for n, f in get_activation_tables(self.m.arch).items()]
    bass_rust.insert_act_table_loads(self, tables)


F32 = mybir.dt.float32
BF16 = mybir.dt.bfloat16
AF = mybir.ActivationFunctionType
OP = mybir.AluOpType

C = 128           # channels == d_inner == partitions
NST = 16          # SSM state dim
NG = 16           # channel groups of 8
RANK = 8          # dt rank
LN_EPS = 1e-5
DCONV = 4
HALO = 4          # halo columns at the left of `un` (col 0 unused, 1..3 conv)

L_FULL = 96 * 96  # 9216

# cols layout: [0]=dt_b [1]=D [2]=cbx [3]=-cbx [4]=zb [5]=-zb [6]=eps
I_DTB, I_D, I_CBX, I_MCBX, I_ZB, I_MZB, I_EPS = range(7)


def build_nc(L, Tc, sub=512):
    assert L % Tc == 0 and Tc % sub == 0
    nchunk = L // Tc
    nsub = Tc // sub

    nc = bacc.Bacc()
    x_in = nc.declare_dram_parameter("x", [C, L], F32, isOutput=False)
    w_inT = nc.declare_dram_parameter("w_inT", [C, 5 * C], BF16, isOutput=False)
    w_xpT = nc.declare_dram_parameter("w_xpT", [C, RANK + 2 * NST], BF16,
                                      isOutput=False)
    w_dtT = nc.declare_dram_parameter("w_dtT", [RANK, C], BF16, isOutput=False)
    w_outT = nc.declare_dram_parameter("w_outT", [C, C], BF16, isOutput=False)
    w_sel = nc.declare_dram_parameter("w_sel", [C, NG * C], BF16,
                                      isOutput=False)
    cols = nc.declare_dram_parameter("cols", [C, 7], F32, isOutput=False)
    a_icols = nc.declare_dram_parameter("a_icols", [C, NG], F32,
                                        isOutput=False)
    y_out = nc.declare_dram_parameter("y", [C, L], F32, isOutput=True)

    with tile.TileContext(nc) as tc, ExitStack() as ctx:
        wpool = ctx.enter_context(tc.tile_pool(name="weights", bufs=1))
        state = ctx.enter_context(tc.tile_pool(name="state", bufs=1))
        io = ctx.enter_context(tc.tile_pool(name="io", bufs=2))
        work = ctx.enter_context(tc.tile_pool(name="work", bufs=2))
        scr = ctx.enter_context(tc.tile_pool(name="scratch", bufs=2))
        scanp = ctx.enter_context(tc.tile_pool(name="scan", bufs=3))
        dram = ctx.enter_context(tc.tile_pool(name="dram", bufs=2,
                                              space="DRAM"))
        ps_st = ctx.enter_context(tc.tile_pool(name="ps_st", bufs=1,
                                               space="PSUM"))
        ps_mm = ctx.enter_context(tc.tile_pool(name="ps_mm", bufs=1,
                                               space="PSUM"))
        ps_y = ctx.enter_context(tc.tile_pool(name="ps_y", bufs=1,
                                              space="PSUM"))

        # ---- weights to SBUF (once) ----
        winT = wpool.tile([C, 5 * C], BF16, tag="winT")
        nc.sync.dma_start(winT[:], w_inT[:])
        wxpT = wpool.tile([C, RANK + 2 * NST], BF16, tag="wxpT")
        nc.sync.dma_start(wxpT[:], w_xpT[:])
        wdtT = wpool.tile([RANK, C], BF16, tag="wdtT")
        nc.sync.dma_start(wdtT[:], w_dtT[:])
        woutT = wpool.tile([C, C], BF16, tag="woutT")
        nc.sync.dma_start(woutT[:], w_outT[:])
        wsel = wpool.tile([C, NG * C], BF16, tag="wsel")
        nc.sync.dma_start(wsel[:], w_sel[:])
        colsb = wpool.tile([C, 7], F32, tag="cols")
        nc.sync.dma_start(colsb[:], cols[:])
        aicol = wpool.tile([C, NG], F32, tag="aicol")
        nc.sync.dma_start(aicol[:], a_icols[:])
        ones_c = wpool.tile([C, C], BF16, tag="ones")
        nc.gpsimd.memset(ones_c[:], 1.0 / C)

        dt_b = colsb[:, I_DTB:I_DTB + 1]
        d_col = colsb[:, I_D:I_D + 1]
        cbx = colsb[:, I_CBX:I_CBX + 1]
        mcbx = colsb[:, I_MCBX:I_MCBX + 1]
        zb = colsb[:, I_ZB:I_ZB + 1]
        mzb = colsb[:, I_MZB:I_MZB + 1]
        eps = colsb[:, I_EPS:I_EPS + 1]

        # ---- persistent scan carries, one per channel-group ----
        carries = []
        for g in range(NG):
            cr = state.tile([C, 1], BF16, tag=f"carry{g}")
            carries.append(cr)

        prev_un = None
        for k in range(nchunk):
            t0 = k * Tc
            xin = io.tile([C, Tc], F32, tag="xin")
            nc.scalar.dma_start(xin[:], x_in[:, t0:t0 + Tc])
            xinbf = io.tile([C, Tc], BF16, tag="xinbf")
            nc.gpsimd.dma_start(xinbf[:], x_in[:, t0:t0 + Tc])

            # ---- LayerNorm over channel (partition) dim, stats via PE ----
            sq = scr.tile([C, Tc], BF16, tag="sq")
            nc.scalar.activation(sq[:], xinbf[:], AF.Square)
            un = work.tile([C, Tc + HALO], BF16, tag="un")
            if k == 0:
                nc.vector.memset(un[:, 1:HALO], 0.0)
            else:
                nc.vector.tensor_copy(un[:, 1:HALO],
                                      prev_un[:, Tc + 1:Tc + HALO])
            prev_un = un
            for j in range(nsub):
                sl = slice(j * sub, (j + 1) * sub)
                mu = ps_st.tile([C, sub], F32, tag="mu")
                nc.tensor.matmul(mu[:], ones_c[:], xinbf[:, sl],
                                 start=True, stop=True)
                m2 = ps_st.tile([C, sub], F32, tag="m2")
                nc.tensor.matmul(m2[:], ones_c[:], sq[:, sl],
                                 start=True, stop=True)
                mubf = scr.tile([C, sub], BF16, tag="mubf")
                nc.scalar.copy(mubf[:], mu[:])
                musq = scr.tile([C, sub], F32, tag="musq")
                nc.scalar.activation(musq[:], mu[:], AF.Square)
                var = scr.tile([C, sub], F32, tag="var")
                nc.vector.tensor_tensor(var[:], m2[:], musq[:], OP.subtract)
                lnv = scr.tile([C, sub], F32, tag="lnv")
                nc.scalar.activation(lnv[:], var[:], AF.Ln, bias=eps)
                rstd = scr.tile([C, sub], BF16, tag="rstd")
                nc.scalar.activation(rstd[:], lnv[:], AF.Exp, scale=-0.5)
                dmu = scr.tile([C, sub], BF16, tag="dmu")
                nc.vector.tensor_tensor(dmu[:], xinbf[:, sl], mubf[:],
                                        OP.subtract)
                nc.vector.tensor_tensor(
                    un[:, HALO + j * sub:HALO + (j + 1) * sub],
                    dmu[:], rstd[:], OP.mult)

            # ---- in_proj + folded causal conv; silu on both branches ----
            zs = work.tile([C, Tc], BF16, tag="zs")
            xs = work.tile([C, Tc], BF16, tag="xs")
            for j in range(nsub):
                sl = slice(j * sub, (j + 1) * sub)
                xm_ps = ps_mm.tile([C, sub], F32, tag="mmA")
                base = HALO - (DCONV - 1) + j * sub  # = 1 + j*sub
                for kk in range(DCONV):
                    nc.tensor.matmul(
                        xm_ps[:], winT[:, kk * C:(kk + 1) * C],
                        un[:, base + kk:base + kk + sub],
                        start=(kk == 0), stop=(kk == DCONV - 1))
                z_ps = ps_mm.tile([C, sub], F32, tag="mmB")
                nc.tensor.matmul(z_ps[:], winT[:, 4 * C:5 * C],
                                 un[:, HALO + j * sub:HALO + j * sub + sub],
                                 start=True, stop=True)
                # silu(z+zb) = (z+zb)*sigmoid(z+zb) via exp/ln1p/exp chain
                es1 = scr.tile([C, sub], F32, tag="es1")
                nc.scalar.activation(es1[:], z_ps[:], AF.Exp, scale=-1.0,
                                     bias=mzb)
                es2 = scr.tile([C, sub], F32, tag="es2")
                nc.scalar.activation(es2[:], es1[:], AF.Ln, bias=1.0)
                sgz = scr.tile([C, sub], F32, tag="sgz")
                nc.scalar.activation(sgz[:], es2[:], AF.Exp, scale=-1.0)
                nc.vector.scalar_tensor_tensor(zs[:, sl], z_ps[:], zb,
                                               sgz[:], OP.add, OP.mult)
                # silu(conv + cbx)
                ec1 = scr.tile([C, sub], F32, tag="ec1")
                nc.scalar.activation(ec1[:], xm_ps[:], AF.Exp, scale=-1.0,
                                     bias=mcbx)
                ec2 = scr.tile([C, sub], F32, tag="ec2")
                nc.scalar.activation(ec2[:], ec1[:], AF.Ln, bias=1.0)
                sgc = scr.tile([C, sub], F32, tag="sgc")
                nc.scalar.activation(sgc[:], ec2[:], AF.Exp, scale=-1.0)
                nc.vector.scalar_tensor_tensor(xs[:, sl], xm_ps[:], cbx,
                                               sgc[:], OP.add, OP.mult)

            # ---- x_proj -> B rows, C rows, dt-rank rows ----
            bc = work.tile([2 * NST, Tc], BF16, tag="bc")
            dtr = work.tile([RANK, Tc], BF16, tag="dtr")
            for j in range(nsub):
                sl = slice(j * sub, (j + 1) * sub)
                dblf = ps_mm.tile([C, sub], F32, tag="mmB")
                dbl = dblf[0:RANK + 2 * NST, :]
                nc.tensor.matmul(dbl, wxpT[:], xs[:, sl],
                                 start=True, stop=True)
                nc.scalar.copy(bc[:, sl], dblf[0:2 * NST, :])
                nc.scalar.copy(dtr[:, sl], dblf[2 * NST:2 * NST + RANK, :])
            bc_d = dram.tile([2 * NST, Tc], BF16, tag="bc_d")
            nc.scalar.dma_start(bc_d[:], bc[:])
            bB = scanp.tile([C, Tc], BF16, tag="bB")
            nc.sync.dma_start(
                bB[:], bc_d[0:NST, :].unsqueeze(1).broadcast_to([NST, 8, Tc]))
            bC = scanp.tile([C, Tc], BF16, tag="bC")
            nc.sync.dma_start(
                bC[:],
                bc_d[NST:2 * NST, :].unsqueeze(1).broadcast_to([NST, 8, Tc]))

            # ---- dt = softplus(dt_proj @ dtr + dt_b) ----
            dt_bf = work.tile([C, Tc], BF16, tag="dt")
            for j in range(nsub):
                sl = slice(j * sub, (j + 1) * sub)
                dt_ps = ps_mm.tile([C, sub], F32, tag="mmA")
                nc.tensor.matmul(dt_ps[:], wdtT[:], dtr[:, sl],
                                 start=True, stop=True)
                spe = scr.tile([C, sub], F32, tag="spe")
                nc.scalar.activation(spe[:], dt_ps[:], AF.Exp, bias=dt_b)
                nc.scalar.activation(dt_bf[:, sl], spe[:], AF.Ln, bias=1.0)

            # ---- v = dt * xs; bounce v, dt to DRAM for block-replication ----
            v_bf = work.tile([C, Tc], BF16, tag="v")
            nc.vector.tensor_tensor(v_bf[:], dt_bf[:], xs[:], OP.mult)
            v_d = dram.tile([C, Tc], BF16, tag="v_d")
            nc.scalar.dma_start(v_d[:], v_bf[:])
            dt_d = dram.tile([C, Tc], BF16, tag="dt_d")
            nc.scalar.dma_start(dt_d[:], dt_bf[:])

            # ---- per-group interleaved scan + PE reduction ----
            y_ps = ps_y.tile([C, Tc], F32, tag="y")
            for g in range(NG):
                dt_rep = scanp.tile([C, Tc], BF16, tag="dt_rep")
                nc.sync.dma_start(
                    dt_rep[:],
                    v_dram_rep(dt_d, g, Tc))
                v_rep = scanp.tile([C, Tc], BF16, tag="v_rep")
                nc.sync.dma_start(
                    v_rep[:],
                    v_dram_rep(v_d, g, Tc))
                dA = scanp.tile([C, Tc], BF16, tag="dA")
                nc.scalar.activation(dA[:], dt_rep[:], AF.Exp,
                                     scale=aicol[:, g:g + 1])
                u = scanp.tile([C, Tc], BF16, tag="u")
                nc.vector.tensor_tensor(u[:], v_rep[:], bB[:], OP.mult)
                h = scanp.tile([C, Tc], BF16, tag="h")
                init = 0.0 if k == 0 else carries[g][:]
                nc.vector.tensor_tensor_scan(h[:], dA[:], u[:], init,
                                             OP.mult, OP.add)
                nc.vector.tensor_copy(carries[g][:], h[:, Tc - 1:Tc])
                gC = scanp.tile([C, Tc], BF16, tag="gC")
                nc.vector.tensor_tensor(gC[:], h[:], bC[:], OP.mult)
                for j in range(nsub):
                    sl = slice(j * sub, (j + 1) * sub)
                    nc.tensor.matmul(y_ps[:, sl],
                                     wsel[:, g * C:(g + 1) * C],
                                     gC[:, sl],
                                     start=(g == 0), stop=(g == NG - 1))

            # ---- gate with zs, D-skip, out_proj, residual ----
            for j in range(nsub):
                sl = slice(j * sub, (j + 1) * sub)
                yD = scr.tile([C, sub], BF16, tag="yD")
                nc.vector.scalar_tensor_tensor(yD[:], xs[:, sl], d_col,
                                               y_ps[:, sl], OP.mult, OP.add)
                yg = scr.tile([C, sub], BF16, tag="yg")
                nc.vector.tensor_tensor(yg[:], yD[:], zs[:, sl], OP.mult)
                o_ps = ps_mm.tile([C, sub], F32, tag="mmO")
                nc.tensor.matmul(o_ps[:], woutT[:], yg[:],
                                 start=True, stop=True)
                ob = io.tile([C, sub], F32, tag="ob")
                nc.vector.tensor_tensor(ob[:], o_ps[:], xin[:, sl], OP.add)
                nc.scalar.dma_start(
                    y_out[:, t0 + j * sub:t0 + (j + 1) * sub], ob[:])

    nc.insert_act_table_loads = types.MethodType(_single_act_table, nc)
    nc.compile()
    return nc


def v_dram_rep(dram_tile, g, Tc):
    """[C, Tc] AP replicating dram rows 8g..8g+7 across the 16 states."""
    return dram_tile[8 * g:8 * g + 8, :].unsqueeze(0).broadcast_to(
        [NST, 8, Tc])


def prep_weights(ln_w, ln_b, in_proj_w, conv_w, conv_b, x_proj_w,
                 dt_proj_w, dt_proj_b, A_log, D, out_proj_w):
    wx = in_proj_w[:C] * ln_w[None, :]       # (out, in) with ln_w folded
    wz = in_proj_w[C:] * ln_w[None, :]
    xb = in_proj_w[:C] @ ln_b                # x-branch const from ln_b
    zb = in_proj_w[C:] @ ln_b
    cbx = conv_b + xb * conv_w.sum(axis=1)
    eps = np.full_like(ln_w, LN_EPS)
    cols = np.stack([dt_proj_b, D, cbx, -cbx, zb, -zb, eps],
                    axis=1).astype(np.float32)
    # selector weights: w_sel[g][p = n*8 + c8, c] = 1 iff c == 8g + c8
    wsel = np.zeros((C, NG * C), np.float32)
    for g in range(NG):
        for n in range(NST):
            for c8 in range(8):
                wsel[n * 8 + c8, g * C + 8 * g + c8] = 1.0
    # A in interleaved layout: a_icols[p = n*8 + c8, g] = -exp(A_log[8g+c8, n])
    A = -np.exp(A_log.astype(np.float32))    # (C, NST)
    aic = np.zeros((C, NG), np.float32)
    for g in range(NG):
        for n in range(NST):
            for c8 in range(8):
                aic[n * 8 + c8, g] = A[8 * g + c8, n]
    # x_proj rows reordered to [B(16), C(16), dt(8)]
    order = list(range(RANK, RANK + 2 * NST)) + list(range(RANK))
    return {
        "w_inT": np.ascontiguousarray(np.concatenate(
            [wx.T * conv_w[:, kk][None, :] for kk in range(DCONV)] + [wz.T],
            axis=1).astype(ml_dtypes.bfloat16)),
        "w_xpT": np.ascontiguousarray(
            x_proj_w[order].T.astype(ml_dtypes.bfloat16)),
        "w_dtT": np.ascontiguousarray(dt_proj_w.T.astype(ml_dtypes.bfloat16)),
        "w_outT": np.ascontiguousarray(
            out_proj_w.T.astype(ml_dtypes.bfloat16)),
        "w_sel": np.ascontiguousarray(wsel.astype(ml_dtypes.bfloat16)),
        "cols": cols,
        "a_icols": aic,
    }


def kernel(input, ln_w, ln_b, in_proj_w, conv_w, conv_b, x_proj_w,
           dt_proj_w, dt_proj_b, A_log, D, out_proj_w, _run=None):
    input = np.asarray(input, np.float32)
    b, c, H, W = input.shape
    L = H * W
    assert c == C and b == 8
    wts = prep_weights(
        np.asarray(ln_w, np.float32), np.asarray(ln_b, np.float32),
        np.asarray(in_proj_w, np.float32), np.asarray(conv_w, np.float32),
        np.asarray(conv_b, np.float32), np.asarray(x_proj_w, np.float32),
        np.asarray(dt_proj_w, np.float32), np.asarray(dt_proj_b, np.float32),
        np.asarray(A_log, np.float32), np.asarray(D, np.float32),
        np.asarray(out_proj_w, np.float32))
    nc = build_nc(L, 1536, 512)
    in_maps = []
    for i in range(8):
        m = {"x": np.ascontiguousarray(input[i].reshape(c, L))}
        m.update(wts)
        in_maps.append(m)
    run = _run or run_bass_kernel_spmd
    res = run(nc, in_maps, core_ids=list(range(8)))
    out = np.stack([np.asarray(res.results[i]["y"]).reshape(c, H, W)
                    for i in range(8)])
    return out.astype(np.float32)


# revision 10
# speedup vs baseline: 1.5609x; 1.0221x over previous
"""CAMMambaBlock Trainium2 kernel, v2 (state-interleaved scan layout).

Data-parallel over batch: 8 batch elements -> 8 NeuronCores. Each core runs
the full block (LayerNorm -> in_proj -> causal depthwise conv -> SiLU ->
x_proj -> dt softplus -> selective scan -> gating -> out_proj -> residual)
on its own (c=128, L=9216) slice, streaming over L in chunks of 1536.

Key layout: the selective scan runs in a state-interleaved layout.  For each
of 16 channel-groups g (8 channels each), a [128, Tc] tile holds all 16 SSM
states: partition p = n*8 + c8 carries the recurrence for (state n, channel
8g+c8).  This makes the per-state B/C coefficient broadcast a single shared
replicated tile per chunk (instead of 16 per-state broadcasts), and turns
the sum over states into PE selector-matmuls that accumulate in PSUM
(instead of a DVE/GpSimd reduction tree).

GpSimd is parked (it shares the DVE's second SBUF port; the v1 kernel's
heavy GpSimd elementwise load degraded DVE scans/TTs by 1.2-2x).  All
elementwise work is DVE in bf16 (2x mode); transcendentals on Scalar;
reductions and projections on the PE.  LayerNorm's ln_w/ln_b and the conv
bias are folded into the in_proj weights host-side.
"""
import types
import numpy as np
import ml_dtypes
from contextlib import ExitStack

import bass_rust

import concourse.bass as bass
import concourse.bacc as bacc
import concourse.tile as tile
from concourse import mybir
from concourse.bass_utils import run_bass_kernel_spmd
from concourse.hw_specs import get_activation_tables


def _single_act_table(self):
    """Force every activation onto natural_log_exp_and_others so the
    table-load pass hoists to one load."""
    if not any(i.opcode == "Activation" for i in self.all_instructions()):
        return
    keep = "natural_log_exp_and_others"
    tables = [(n, (f if n == keep else set()))
              for n, f in get_activation_tables(self.m.arch).items()]
    bass_rust.insert_act_table_loads(self, tables)


F32 = mybir.dt.float32
BF16 = mybir.dt.bfloat16
AF = mybir.ActivationFunctionType
OP = mybir.AluOpType

C = 128           # channels == d_inner == partitions
NST = 16          # SSM state dim
NG = 16           # channel groups of 8
RANK = 8          # dt rank
LN_EPS = 1e-5
DCONV = 4
HALO = 4          # halo columns at the left of `un` (col 0 unused, 1..3 conv)

L_FULL = 96 * 96  # 9216

# cols layout: [0]=dt_b [1]=D [2]=cbx [3]=-cbx [4]=zb [5]=-zb [6]=eps
I_DTB, I_D, I_CBX, I_MCBX, I_ZB, I_MZB, I_EPS = range(7)


def build_nc(L, Tc, sub=512):
    assert L % Tc == 0 and Tc % sub == 0
    nchunk = L // Tc
    nsub = Tc // sub

    nc = bacc.Bacc()
    x_in = nc.declare_dram_parameter("x", [C, L], F32, isOutput=False)
    w_inT = nc.declare_dram_parameter("w_inT", [C, 5 * C], BF16, isOutput=False)
    w_xpT = nc.declare_dram_parameter("w_xpT", [C, RANK + 2 * NST], BF16,
                                      isOutput=False)
    w_dtT = nc.declare_dram_parameter("w_dtT", [RANK, C], BF16, isOutput=False)
    w_outT = nc.declare_dram_parameter("w_outT", [C, C], BF16, isOutput=False)
    w_sel = nc.declare_dram_parameter("w_sel", [C, NG * C], BF16,
                                      isOutput=False)
    cols = nc.declare_dram_parameter("cols", [C, 7], F32, isOutput=False)
    a_icols = nc.declare_dram_parameter("a_icols", [C, NG], F32,
                                        isOutput=False)
    y_out = nc.declare_dram_parameter("y", [C, L], F32, isOutput=True)

    with tile.TileContext(nc) as tc, ExitStack() as ctx:
        wpool = ctx.enter_context(tc.tile_pool(name="weights", bufs=1))
        state = ctx.enter_context(tc.tile_pool(name="state", bufs=1))
        io = ctx.enter_context(tc.tile_pool(name="io", bufs=2))
        work = ctx.enter_context(tc.tile_pool(name="work", bufs=2))
        scr = ctx.enter_context(tc.tile_pool(name="scratch", bufs=2))
        reps = ctx.enter_context(tc.tile_pool(name="reps", bufs=5))
        scanp = ctx.enter_context(tc.tile_pool(name="scan", bufs=3))
        bbp = ctx.enter_context(tc.tile_pool(name="bb", bufs=2))
        dram = ctx.enter_context(tc.tile_pool(name="dram", bufs=2,
                                              space="DRAM"))
        ps_st = ctx.enter_context(tc.tile_pool(name="ps_st", bufs=1,
                                               space="PSUM"))
        ps_mm = ctx.enter_context(tc.tile_pool(name="ps_mm", bufs=1,
                                               space="PSUM"))
        ps_y = ctx.enter_context(tc.tile_pool(name="ps_y", bufs=1,
                                              space="PSUM"))

        # ---- weights to SBUF (once) ----
        winT = wpool.tile([C, 5 * C], BF16, tag="winT")
        nc.sync.dma_start(winT[:], w_inT[:])
        wxpT = wpool.tile([C, RANK + 2 * NST], BF16, tag="wxpT")
        nc.sync.dma_start(wxpT[:], w_xpT[:])
        wdtT = wpool.tile([RANK, C], BF16, tag="wdtT")
        nc.sync.dma_start(wdtT[:], w_dtT[:])
        woutT = wpool.tile([C, C], BF16, tag="woutT")
        nc.sync.dma_start(woutT[:], w_outT[:])
        wsel = wpool.tile([C, NG * C], BF16, tag="wsel")
        nc.sync.dma_start(wsel[:], w_sel[:])
        colsb = wpool.tile([C, 7], F32, tag="cols")
        nc.sync.dma_start(colsb[:], cols[:])
        aicol = wpool.tile([C, NG], F32, tag="aicol")
        nc.sync.dma_start(aicol[:], a_icols[:])
        ones_c = wpool.tile([C, C], BF16, tag="ones")
        nc.gpsimd.memset(ones_c[:], 1.0 / C)

        dt_b = colsb[:, I_DTB:I_DTB + 1]
        d_col = colsb[:, I_D:I_D + 1]
        cbx = colsb[:, I_CBX:I_CBX + 1]
        mcbx = colsb[:, I_MCBX:I_MCBX + 1]
        zb = colsb[:, I_ZB:I_ZB + 1]
        mzb = colsb[:, I_MZB:I_MZB + 1]
        eps = colsb[:, I_EPS:I_EPS + 1]

        # ---- persistent scan carries, one per channel-group ----
        carries = []
        for g in range(NG):
            cr = state.tile([C, 1], BF16, tag=f"carry{g}")
            carries.append(cr)

        prev_un = None
        for k in range(nchunk):
            t0 = k * Tc
            xin = io.tile([C, Tc], F32, tag="xin")
            nc.sync.dma_start(xin[:], x_in[:, t0:t0 + Tc])
            xinbf = io.tile([C, Tc], BF16, tag="xinbf")
            nc.gpsimd.dma_start(xinbf[:], x_in[:, t0:t0 + Tc])

            # ---- LayerNorm over channel (partition) dim, stats via PE ----
            sq = scr.tile([C, Tc], BF16, tag="sq")
            nc.scalar.activation(sq[:], xinbf[:], AF.Square)
            un = work.tile([C, Tc + HALO], BF16, tag="un")
            if k == 0:
                nc.vector.memset(un[:, 1:HALO], 0.0)
            else:
                nc.vector.tensor_copy(un[:, 1:HALO],
                                      prev_un[:, Tc + 1:Tc + HALO])
            prev_un = un
            for j in range(nsub):
                sl = slice(j * sub, (j + 1) * sub)
                mu = ps_st.tile([C, sub], F32, tag="mu")
                nc.tensor.matmul(mu[:], ones_c[:], xinbf[:, sl],
                                 start=True, stop=True)
                m2 = ps_st.tile([C, sub], F32, tag="m2")
                nc.tensor.matmul(m2[:], ones_c[:], sq[:, sl],
                                 start=True, stop=True)
                mubf = scr.tile([C, sub], BF16, tag="mubf")
                nc.scalar.copy(mubf[:], mu[:])
                musq = scr.tile([C, sub], F32, tag="musq")
                nc.scalar.activation(musq[:], mu[:], AF.Square)
                var = scr.tile([C, sub], F32, tag="var")
                nc.vector.tensor_tensor(var[:], m2[:], musq[:], OP.subtract)
                lnv = scr.tile([C, sub], F32, tag="lnv")
                nc.scalar.activation(lnv[:], var[:], AF.Ln, bias=eps)
                rstd = scr.tile([C, sub], BF16, tag="rstd")
                nc.scalar.activation(rstd[:], lnv[:], AF.Exp, scale=-0.5)
                dmu = scr.tile([C, sub], BF16, tag="dmu")
                nc.vector.tensor_tensor(dmu[:], xinbf[:, sl], mubf[:],
                                        OP.subtract)
                nc.vector.tensor_tensor(
                    un[:, HALO + j * sub:HALO + (j + 1) * sub],
                    dmu[:], rstd[:], OP.mult)

            # ---- in_proj + folded causal conv; silu on both branches ----
            zs = work.tile([C, Tc], BF16, tag="zs")
            xs = work.tile([C, Tc], BF16, tag="xs")
            for j in range(nsub):
                sl = slice(j * sub, (j + 1) * sub)
                xm_ps = ps_mm.tile([C, sub], F32, tag="mmA")
                base = HALO - (DCONV - 1) + j * sub  # = 1 + j*sub
                for kk in range(DCONV):
                    nc.tensor.matmul(
                        xm_ps[:], winT[:, kk * C:(kk + 1) * C],
                        un[:, base + kk:base + kk + sub],
                        start=(kk == 0), stop=(kk == DCONV - 1))
                z_ps = ps_mm.tile([C, sub], F32, tag="mmB")
                nc.tensor.matmul(z_ps[:], winT[:, 4 * C:5 * C],
                                 un[:, HALO + j * sub:HALO + j * sub + sub],
                                 start=True, stop=True)
                # silu(z+zb) = (z+zb)*sigmoid(z+zb) via exp/ln1p/exp chain
                es1 = scr.tile([C, sub], F32, tag="es1")
                nc.scalar.activation(es1[:], z_ps[:], AF.Exp, scale=-1.0,
                                     bias=mzb)
                es2 = scr.tile([C, sub], F32, tag="es2")
                nc.scalar.activation(es2[:], es1[:], AF.Ln, bias=1.0)
                sgz = scr.tile([C, sub], F32, tag="sgz")
                nc.scalar.activation(sgz[:], es2[:], AF.Exp, scale=-1.0)
                nc.vector.scalar_tensor_tensor(zs[:, sl], z_ps[:], zb,
                                               sgz[:], OP.add, OP.mult)
                # silu(conv + cbx)
                ec1 = scr.tile([C, sub], F32, tag="ec1")
                nc.scalar.activation(ec1[:], xm_ps[:], AF.Exp, scale=-1.0,
                                     bias=mcbx)
                ec2 = scr.tile([C, sub], F32, tag="ec2")
                nc.scalar.activation(ec2[:], ec1[:], AF.Ln, bias=1.0)
                sgc = scr.tile([C, sub], F32, tag="sgc")
                nc.scalar.activation(sgc[:], ec2[:], AF.Exp, scale=-1.0)
                nc.vector.scalar_tensor_tensor(xs[:, sl], xm_ps[:], cbx,
                                               sgc[:], OP.add, OP.mult)

            # ---- x_proj -> B rows, C rows, dt-rank rows ----
            bc = work.tile([2 * NST, Tc], BF16, tag="bc")
            dtr = work.tile([RANK, Tc], BF16, tag="dtr")
            for j in range(nsub):
                sl = slice(j * sub, (j + 1) * sub)
                dblf = ps_mm.tile([C, sub], F32, tag="mmB")
                dbl = dblf[0:RANK + 2 * NST, :]
                nc.tensor.matmul(dbl, wxpT[:], xs[:, sl],
                                 start=True, stop=True)
                nc.scalar.copy(bc[:, sl], dblf[0:2 * NST, :])
                nc.scalar.copy(dtr[:, sl], dblf[2 * NST:2 * NST + RANK, :])
            bc_d = dram.tile([2 * NST, Tc], BF16, tag="bc_d")
            nc.sync.dma_start(bc_d[:], bc[:])
            bB = bbp.tile([C, Tc], BF16, tag="bB")
            nc.sync.dma_start(
                bB[:], bc_d[0:NST, :].unsqueeze(1).broadcast_to([NST, 8, Tc]))
            bC = bbp.tile([C, Tc], BF16, tag="bC")
            nc.sync.dma_start(
                bC[:],
                bc_d[NST:2 * NST, :].unsqueeze(1).broadcast_to([NST, 8, Tc]))

            # ---- dt = softplus(dt_proj @ dtr + dt_b); v = dt * xs ----
            # per-sub DRAM writes so the replication reads can start early
            dt_bf = work.tile([C, Tc], BF16, tag="dt")
            v_bf = work.tile([C, Tc], BF16, tag="v")
            dt_d = dram.tile([C, Tc], BF16, tag="dt_d")
            v_d = dram.tile([C, Tc], BF16, tag="v_d")
            for j in range(nsub):
                sl = slice(j * sub, (j + 1) * sub)
                dt_ps = ps_mm.tile([C, sub], F32, tag="mmA")
                nc.tensor.matmul(dt_ps[:], wdtT[:], dtr[:, sl],
                                 start=True, stop=True)
                spe = scr.tile([C, sub], F32, tag="spe")
                nc.scalar.activation(spe[:], dt_ps[:], AF.Exp, bias=dt_b)
                nc.scalar.activation(dt_bf[:, sl], spe[:], AF.Ln, bias=1.0)
                nc.sync.dma_start(dt_d[:, sl], dt_bf[:, sl])
                nc.vector.tensor_tensor(v_bf[:, sl], dt_bf[:, sl], xs[:, sl],
                                        OP.mult)
                nc.sync.dma_start(v_d[:, sl], v_bf[:, sl])

            # ---- per-group interleaved scan + PE reduction ----
            y_ps = ps_y.tile([C, Tc], F32, tag="y")
            for g in range(NG):
                dt_rep = reps.tile([C, Tc], BF16, tag="dt_rep")
                nc.sync.dma_start(
                    dt_rep[:],
                    v_dram_rep(dt_d, g, Tc))
                v_rep = reps.tile([C, Tc], BF16, tag="v_rep")
                nc.gpsimd.dma_start(
                    v_rep[:],
                    v_dram_rep(v_d, g, Tc))
                dA = scanp.tile([C, Tc], BF16, tag="dA")
                nc.scalar.activation(dA[:], dt_rep[:], AF.Exp,
                                     scale=aicol[:, g:g + 1])
                u = scanp.tile([C, Tc], BF16, tag="u")
                nc.vector.tensor_tensor(u[:], v_rep[:], bB[:], OP.mult)
                h = scanp.tile([C, Tc], BF16, tag="h")
                init = 0.0 if k == 0 else carries[g][:]
                nc.vector.tensor_tensor_scan(h[:], dA[:], u[:], init,
                                             OP.mult, OP.add)
                nc.vector.tensor_copy(carries[g][:], h[:, Tc - 1:Tc])
                gC = scanp.tile([C, Tc], BF16, tag="gC")
                nc.vector.tensor_tensor(gC[:], h[:], bC[:], OP.mult)
                for j in range(nsub):
                    sl = slice(j * sub, (j + 1) * sub)
                    nc.tensor.matmul(y_ps[:, sl],
                                     wsel[:, g * C:(g + 1) * C],
                                     gC[:, sl],
                                     start=(g == 0), stop=(g == NG - 1))

            # ---- gate with zs, D-skip, out_proj, residual ----
            for j in range(nsub):
                sl = slice(j * sub, (j + 1) * sub)
                yD = scr.tile([C, sub], BF16, tag="yD")
                nc.vector.scalar_tensor_tensor(yD[:], xs[:, sl], d_col,
                                               y_ps[:, sl], OP.mult, OP.add)
                yg = scr.tile([C, sub], BF16, tag="yg")
                nc.vector.tensor_tensor(yg[:], yD[:], zs[:, sl], OP.mult)
                o_ps = ps_mm.tile([C, sub], F32, tag="mmO")
                nc.tensor.matmul(o_ps[:], woutT[:], yg[:],
                                 start=True, stop=True)
                ob = io.tile([C, sub], F32, tag="ob")
                nc.vector.tensor_tensor(ob[:], o_ps[:], xin[:, sl], OP.add)
                nc.gpsimd.dma_start(
                    y_out[:, t0 + j * sub:t0 + (j + 1) * sub], ob[:])

    nc.insert_act_table_loads = types.MethodType(_single_act_table, nc)
    nc.compile()
    return nc


def v_dram_rep(dram_tile, g, Tc):
    """[C, Tc] AP replicating dram rows 8g..8g+7 across the 16 states."""
    return dram_tile[8 * g:8 * g + 8, :].unsqueeze(0).broadcast_to(
        [NST, 8, Tc])


def prep_weights(ln_w, ln_b, in_proj_w, conv_w, conv_b, x_proj_w,
                 dt_proj_w, dt_proj_b, A_log, D, out_proj_w):
    wx = in_proj_w[:C] * ln_w[None, :]       # (out, in) with ln_w folded
    wz = in_proj_w[C:] * ln_w[None, :]
    xb = in_proj_w[:C] @ ln_b                # x-branch const from ln_b
    zb = in_proj_w[C:] @ ln_b
    cbx = conv_b + xb * conv_w.sum(axis=1)
    eps = np.full_like(ln_w, LN_EPS)
    cols = np.stack([dt_proj_b, D, cbx, -cbx, zb, -zb, eps],
                    axis=1).astype(np.float32)
    # selector weights: w_sel[g][p = n*8 + c8, c] = 1 iff c == 8g + c8
    wsel = np.zeros((C, NG * C), np.float32)
    for g in range(NG):
        for n in range(NST):
            for c8 in range(8):
                wsel[n * 8 + c8, g * C + 8 * g + c8] = 1.0
    # A in interleaved layout: a_icols[p = n*8 + c8, g] = -exp(A_log[8g+c8, n])
    A = -np.exp(A_log.astype(np.float32))    # (C, NST)
    aic = np.zeros((C, NG), np.float32)
    for g in range(NG):
        for n in range(NST):
            for c8 in range(8):
                aic[n * 8 + c8, g] = A[8 * g + c8, n]
    # x_proj rows reordered to [B(16), C(16), dt(8)]
    order = list(range(RANK, RANK + 2 * NST)) + list(range(RANK))
    return {
        "w_inT": np.ascontiguousarray(np.concatenate(
            [wx.T * conv_w[:, kk][None, :] for kk in range(DCONV)] + [wz.T],
            axis=1).astype(ml_dtypes.bfloat16)),
        "w_xpT": np.ascontiguousarray(
            x_proj_w[order].T.astype(ml_dtypes.bfloat16)),
        "w_dtT": np.ascontiguousarray(dt_proj_w.T.astype(ml_dtypes.bfloat16)),
        "w_outT": np.ascontiguousarray(
            out_proj_w.T.astype(ml_dtypes.bfloat16)),
        "w_sel": np.ascontiguousarray(wsel.astype(ml_dtypes.bfloat16)),
        "cols": cols,
        "a_icols": aic,
    }


def kernel(input, ln_w, ln_b, in_proj_w, conv_w, conv_b, x_proj_w,
           dt_proj_w, dt_proj_b, A_log, D, out_proj_w, _run=None):
    input = np.asarray(input, np.float32)
    b, c, H, W = input.shape
    L = H * W
    assert c == C and b == 8
    wts = prep_weights(
        np.asarray(ln_w, np.float32), np.asarray(ln_b, np.float32),
        np.asarray(in_proj_w, np.float32), np.asarray(conv_w, np.float32),
        np.asarray(conv_b, np.float32), np.asarray(x_proj_w, np.float32),
        np.asarray(dt_proj_w, np.float32), np.asarray(dt_proj_b, np.float32),
        np.asarray(A_log, np.float32), np.asarray(D, np.float32),
        np.asarray(out_proj_w, np.float32))
    nc = build_nc(L, 1536, 512)
    in_maps = []
    for i in range(8):
        m = {"x": np.ascontiguousarray(input[i].reshape(c, L))}
        m.update(wts)
        in_maps.append(m)
    run = _run or run_bass_kernel_spmd
    res = run(nc, in_maps, core_ids=list(range(8)))
    out = np.stack([np.asarray(res.results[i]["y"]).reshape(c, H, W)
                    for i in range(8)])
    return out.astype(np.float32)
